# revision 3
# baseline (speedup 1.0000x reference)
"""CFNO forward kernel for Trainium2 (8 NeuronCores, data-parallel over batch).

The reference computes, per 16x16 patch p (flattened to 256):
    fft = FFT_256(p) (ortho); fc = fft @ Wc^T + bc; y = Re(IFFT_16(fc)) (ortho)
    z = y @ conv_w^T + conv_b;  out = GroupNorm_8(z) * gamma + beta

Because p is real and every step before GroupNorm is linear, the whole chain
folds into one real matrix on the host:
    M2 = Re(F @ Wc^T @ G) @ conv_w^T   [256, 16]
    b2 = Re(bc @ G) @ conv_w^T + conv_b [16]
    z  = p @ M2 + b2
(F = symmetric 256-pt DFT matrix / sqrt(256); G = inverse 16-pt DFT / sqrt(16))

On-device per core (one batch image, x [2048, 2048]):
  - 16 row-blocks of 128 image rows; SBUF layout [128 part=(hblk, s1), 2048]
  - per block, 16 PSUM-accumulating matmuls (one per patch-column offset s2,
    free dim 128) with a block-diagonal lhsT so all 8 h-blocks share a matmul;
    float32r keeps full fp32 storage with a fast (TF32-like) PE mode
  - z stays in PSUM (4 banks hold all 16 blocks); bn_stats reads PSUM; the
    fc bias b2 is folded into the final normalize coefficients
  - one mask-matmul does the grouped cross-partition reduce AND broadcast
  - normalize (z*A + B) fused with the PSUM->SBUF move, chunked, with the
    output DMA of each chunk overlapping the next chunk's normalize

DMA descriptor scheduling: descriptors of each dma_start round-robin over
the 16 SDMA engines starting at engine 0.  Engine 15 is ~17% slower than
the rest (known HW quirk), so every full row-block DMA is split into
partitions [0:127] (127 descs -> engine 15 gets 7 instead of 8) plus
partition 127's row as 8x1KB descs (engines 0-7, +1KB each).  This keeps
all 16 engines finishing together instead of engine 15 straggling ~9us.
"""

import numpy as np
from contextlib import ExitStack

CHUNK = 16
GROUPS = 8
EPS = 1e-5
B, C, H, W = 8, 1, 2048, 2048
D = 16
D_IN = CHUNK * CHUNK * C  # 256
HP = H // CHUNK  # 128 patch rows
WP = W // CHUNK  # 128 patch cols
P = 128
RB = 16  # 128-row blocks per image
N_CORES = 8

_CACHED_NC = {}


def _build_nc(mm_dtype="float32r"):
    import concourse.bass as bass
    import concourse.tile as tile
    from concourse import bacc, mybir

    f32 = mybir.dt.float32
    mmdt = getattr(mybir.dt, mm_dtype)
    nc = bacc.Bacc("TRN2", target_bir_lowering=False, debug=False,
                   num_devices=N_CORES)

    x = nc.dram_tensor("x", [H, W], mmdt, kind="ExternalInput").ap()
    # host-packed [p, s2, m] so the SBUF load is contiguous per partition
    wl = nc.dram_tensor("wl", [P, CHUNK * P], mmdt, kind="ExternalInput").ap()
    gmask = nc.dram_tensor("gmask", [P, P], f32, kind="ExternalInput").ap()
    consts = nc.dram_tensor("consts", [P, 3], f32, kind="ExternalInput").ap()
    # [p=(hblk,e), rg, w] flattened (rg = hi//8); host reorders to [D, HP, WP]
    out = nc.dram_tensor("out", [P, RB * WP], f32, kind="ExternalOutput").ap()

    Ident = mybir.ActivationFunctionType.Identity
    Sqrt = mybir.ActivationFunctionType.Sqrt
    Mult = mybir.AluOpType.mult
    Add = mybir.AluOpType.add
    Sub = mybir.AluOpType.subtract

    with tile.TileContext(nc) as tc, ExitStack() as ctx:
        const_pool = ctx.enter_context(tc.tile_pool(name="const", bufs=1))
        xin = ctx.enter_context(tc.tile_pool(name="xin", bufs=4))
        zpool = ctx.enter_context(tc.tile_pool(name="z", bufs=1))
        # 4 persistent PSUM banks hold z for all 16 blocks; 1 more for gp
        zpsum = ctx.enter_context(tc.tile_pool(name="zp", bufs=4, space="PSUM"))
        psg = ctx.enter_context(tc.tile_pool(name="psg", bufs=1, space="PSUM"))

        # x row-block rb covers image rows [rb*128, (rb+1)*128):
        # row = rb*128 + p, p = (hblk, s1)
        xr = x.rearrange("(rb p) c -> rb p c", rb=RB, p=P)

        def dma_block(eng, xt, rb):
            # engine-15 relief: 127 full descs + 8 small descs on engines 0-7
            eng.dma_start(out=xt[0:127], in_=xr[rb, 0:127])
            eng.dma_start(
                out=xt[127:128].rearrange("p (c k) -> p c k", c=8),
                in_=xr[rb, 127:128].rearrange("p (c k) -> p c k", c=8))

        # stream: even blocks on SP ring, odd blocks + weights on ACT ring
        xts = []
        for rb in range(RB):
            eng = nc.sync if rb % 2 == 0 else nc.scalar
            xt = xin.tile([P, W], mmdt, tag="xt", name=f"xt{rb}")
            if rb == RB - 1:
                # last block in two w-halves so its matmuls start earlier
                h = W // 2
                eng.dma_start(out=xt[:, :h], in_=xr[rb, :, :h])
                eng.dma_start(out=xt[:, h:], in_=xr[rb, :, h:])
            else:
                dma_block(eng, xt, rb)
            xts.append(xt)
            if rb == 0:
                wtile = const_pool.tile([P, CHUNK, P], mmdt)
                wv = wtile.rearrange("p s m -> p (s m)")
                nc.scalar.dma_start(out=wv[0:127], in_=wl[0:127])
                nc.scalar.dma_start(
                    out=wv[127:128].rearrange("p (c k) -> p c k", c=8),
                    in_=wl[127:128].rearrange("p (c k) -> p c k", c=8))
                gmt = const_pool.tile([P, P], f32)
                nc.scalar.dma_start(out=gmt, in_=gmask)
                cvt = const_pool.tile([P, 3], f32)
                nc.scalar.dma_start(out=cvt, in_=consts)
                epst = const_pool.tile([P, 1], f32)
                nc.vector.memset(epst, EPS)
                # touch Sqrt (exact scale/bias shape used later) early so its
                # ACT table loads during the stream, not in the stats chain
                warm = const_pool.tile([P, 1], f32)
                nc.scalar.activation(out=warm, in_=epst, func=Sqrt,
                                     bias=epst, scale=-1.0)
                nc.scalar.activation(out=warm, in_=epst, func=Ident,
                                     bias=epst, scale=epst)

        nstats = RB + 1  # last block contributes two half-stats
        statsall = zpool.tile([P, nstats, nc.vector.BN_STATS_DIM], f32)
        zts = [zpsum.tile([P, 4, WP], f32, tag="zt", name=f"zt{g}")
               for g in range(4)]

        for rb in range(RB):
            xt = xts[rb]
            xs = xt.rearrange("p (w s) -> p w s", s=CHUNK)
            pt = zts[rb // 4][:, rb % 4]
            if rb == RB - 1:
                hw = WP // 2
                for half in range(2):
                    ws = slice(half * hw, (half + 1) * hw)
                    for s2 in range(CHUNK):
                        nc.tensor.matmul(pt[:, ws], lhsT=wtile[:, s2, :],
                                         rhs=xs[:, ws, s2],
                                         start=(s2 == 0),
                                         stop=(s2 == CHUNK - 1))
                    nc.vector.bn_stats(out=statsall[:, rb + half],
                                       in_=pt[:, ws])
            else:
                for s2 in range(CHUNK):
                    nc.tensor.matmul(pt, lhsT=wtile[:, s2, :],
                                     rhs=xs[:, :, s2],
                                     start=(s2 == 0), stop=(s2 == CHUNK - 1))
                nc.vector.bn_stats(out=statsall[:, rb], in_=pt)

        # Per-partition raw mean'/var over all 2048 elements; fc bias b2 is
        # folded in here (z_true = raw + b2) and into the normalize offset.
        mv = zpool.tile([P, 2], f32)
        nc.vector.bn_aggr(out=mv, in_=statsall)
        # me2 = (-mean, E2) of biased z per partition:
        #   negmean = -(mean' + b2);  E2 = negmean^2 + var
        me2 = zpool.tile([P, 2], f32)
        nc.vector.tensor_scalar(out=me2[:, 0:1], in0=mv[:, 0:1],
                                scalar1=cvt[:, 0:1], scalar2=-1.0,
                                op0=Add, op1=Mult)
        nc.vector.scalar_tensor_tensor(
            out=me2[:, 1:2], in0=me2[:, 0:1], scalar=me2[:, 0:1],
            in1=mv[:, 1:2], op0=Mult, op1=Add)
        # Grouped cross-partition average + broadcast in one matmul:
        # gp[p'] = (1/16) * sum_{p in group(p')} me2[p] = (-mean_g, E2_g)
        gp = psg.tile([P, 2], f32)
        nc.tensor.matmul(gp, lhsT=gmt, rhs=me2, start=True, stop=True)
        gsb = zpool.tile([P, 2], f32)
        nc.vector.tensor_copy(gsb, gp)
        # negvar = mean_g^2 - E2_g;  sd = sqrt(-negvar + eps)
        negvar = zpool.tile([P, 1], f32)
        nc.vector.scalar_tensor_tensor(
            out=negvar, in0=gsb[:, 0:1], scalar=gsb[:, 0:1], in1=gsb[:, 1:2],
            op0=Mult, op1=Sub)
        sd = zpool.tile([P, 1], f32)
        nc.scalar.activation(out=sd, in_=negvar, func=Sqrt, bias=epst,
                             scale=-1.0)
        # v = b2 - mean_g (overlaps with Sqrt on the ACT engine)
        v = zpool.tile([P, 1], f32)
        nc.vector.tensor_add(v, cvt[:, 0:1], gsb[:, 0:1])
        rs = zpool.tile([P, 1], f32)
        nc.vector.reciprocal(rs, sd)
        # out = raw*A + B2 with A = rsqrt*gamma, B2 = (b2 - mean_g)*A + beta
        A = zpool.tile([P, 1], f32)
        nc.vector.tensor_mul(A, rs, cvt[:, 1:2])
        B2 = zpool.tile([P, 1], f32)
        nc.vector.scalar_tensor_tensor(
            out=B2, in0=A, scalar=v, in1=cvt[:, 2:3], op0=Mult, op1=Add)

        # normalize PSUM->SBUF in 4 chunks (ACT and DVE split them), each
        # chunk's output DMA overlapping the next chunk's normalize
        onorm = zpool.tile([P, RB * WP], f32)
        for g in range(4):
            sl = slice(g * 4 * WP, (g + 1) * 4 * WP)
            zin = zts[g].rearrange("p a b -> p (a b)")
            if g % 2 == 0:
                nc.vector.tensor_scalar(out=onorm[:, sl], in0=zin,
                                        scalar1=A, scalar2=B2,
                                        op0=Mult, op1=Add)
            else:
                nc.scalar.activation(out=onorm[:, sl], in_=zin,
                                     func=Ident, scale=A, bias=B2)
            eng = nc.sync if g % 2 == 0 else nc.scalar
            eng.dma_start(out=out[:, sl], in_=onorm[:, sl])

    nc.compile()
    return nc


def _host_weights(fc_wr, fc_wi, fc_br, fc_bi, conv_w, conv_b, gamma, beta):
    fc_wr = np.asarray(fc_wr, np.float64)
    fc_wi = np.asarray(fc_wi, np.float64)
    fc_br = np.asarray(fc_br, np.float64)
    fc_bi = np.asarray(fc_bi, np.float64)
    conv_w = np.asarray(conv_w, np.float64)
    conv_b = np.asarray(conv_b, np.float64)
    gamma = np.asarray(gamma, np.float64)
    beta = np.asarray(beta, np.float64)

    j = np.arange(D_IN)
    F = np.exp(-2j * np.pi * np.outer(j, j) / D_IN) / np.sqrt(D_IN)
    d = np.arange(D)
    G = np.exp(2j * np.pi * np.outer(d, d) / D) / np.sqrt(D)
    Wc = fc_wr + 1j * fc_wi
    bc = fc_br + 1j * fc_bi
    M2 = (np.real(F @ Wc.T @ G) @ conv_w.T).astype(np.float32)  # [256, 16]
    b2 = (np.real(bc @ G) @ conv_w.T + conv_b).astype(np.float32)  # [16]

    # Block-diagonal lhsT: wl[hblk*16+s1, s2, hblk*16+e] = M2[s1*16+s2, e],
    # packed [p, s2*128+m] for a contiguous per-partition SBUF load.
    wl = np.zeros((CHUNK, P, P), np.float32)  # [s2, p, m]
    blk = M2.reshape(CHUNK, CHUNK, D).transpose(1, 0, 2)  # [s2, s1, e]
    for hb in range(8):
        wl[:, hb * 16:hb * 16 + 16, hb * 16:hb * 16 + 16] = blk
    wl = np.ascontiguousarray(wl.transpose(1, 0, 2).reshape(P, CHUNK * P))

    # Group-average + broadcast mask; each partition holds 2048 elements,
    # each group spans 16 partitions -> scale 1/16 on the per-partition means
    pidx = np.arange(P)
    grp = (pidx % D) // (D // GROUPS)
    gmask = (grp[:, None] == grp[None, :]).astype(np.float32) / 16.0

    e = pidx % D
    consts = np.stack([b2[e], gamma.astype(np.float32)[e],
                       beta.astype(np.float32)[e]], axis=1)  # [128, 3]
    return wl, gmask, consts


def kernel(x, fc_wr, fc_wi, fc_br, fc_bi, conv_w, conv_b, gamma, beta,
           _return_results=False, _trace=False, _mm_dtype="float32r"):
    from concourse.bass_utils import run_bass_kernel_spmd

    if _mm_dtype not in _CACHED_NC:
        _CACHED_NC[_mm_dtype] = _build_nc(_mm_dtype)
    nc = _CACHED_NC[_mm_dtype]

    wl, gmask, consts = _host_weights(fc_wr, fc_wi, fc_br, fc_bi,
                                      conv_w, conv_b, gamma, beta)
    x = np.ascontiguousarray(np.asarray(x, np.float32).reshape(B, H, W))
    in_maps = [{"x": x[b], "wl": wl, "gmask": gmask, "consts": consts}
               for b in range(N_CORES)]
    res = run_bass_kernel_spmd(nc, in_maps, list(range(N_CORES)),
                               trace=_trace)
    # device layout [p=(hblk,e), rg, w] -> [D, HP, WP], hi = rg*8 + hblk
    out = np.stack(
        [res.results[b]["out"].reshape(8, D, RB, WP)
         .transpose(1, 2, 0, 3).reshape(D, HP, WP)
         for b in range(N_CORES)], axis=0)
    if _return_results:
        return out, res
    return out


# revision 5
# speedup vs baseline: 6.0243x; 6.0243x over previous
"""CFNO forward kernel for Trainium2 (8 NeuronCores, data-parallel over batch).

The reference computes, per 16x16 patch p (flattened to 256):
    fft = FFT_256(p) (ortho); fc = fft @ Wc^T + bc; y = Re(IFFT_16(fc)) (ortho)
    z = y @ conv_w^T + conv_b;  out = GroupNorm_8(z) * gamma + beta

Because p is real and every step before GroupNorm is linear, the whole chain
folds into one real matrix on the host:
    M2 = Re(F @ Wc^T @ G) @ conv_w^T   [256, 16]
    b2 = Re(bc @ G) @ conv_w^T + conv_b [16]
    z  = p @ M2 + b2
(F = symmetric 256-pt DFT matrix / sqrt(256); G = inverse 16-pt DFT / sqrt(16))

On-device per core (one batch image, x [2048, 2048]):
  - 16 row-blocks of 128 image rows; SBUF layout [128 part=(hblk, s1), 2048]
  - per block, 16 PSUM-accumulating matmuls (one per patch-column offset s2,
    free dim 128) with a block-diagonal lhsT so all 8 h-blocks share a matmul;
    float32r keeps full fp32 storage with a fast (TF32-like) PE mode
  - z stays in PSUM (4 banks hold all 16 blocks); bn_stats reads PSUM; the
    fc bias b2 is folded into the final normalize coefficients
  - one mask-matmul does the grouped cross-partition reduce AND broadcast
  - normalize (z*A + B) fused with the PSUM->SBUF move, chunked, with the
    output DMA of each chunk overlapping the next chunk's normalize

DMA descriptor scheduling (probed on HW): each dma_start's descriptors are
distributed over the 16 SDMA engines by descriptor index (engine = idx mod
16 for strided APs; nice contiguous [128, N] blocks are sprayed as 16
chunks of 8 partitions).  Partition counts not divisible by 16 collapse
onto a single engine (avoid!).  A transfer with only 8 descriptors lands on
engines 0-7 only.  Engine 15 is ~17% slower than the rest (known HW
quirk), so 12 of the 16 row-block DMAs are split [0:112] + [112:120] +
[120:128]: the two 8-desc transfers land on engines 0-7, shifting ~8KB per
block off engines 8-15.  Mixed with 4 uniform blocks this makes all 16
engines finish together instead of engine 15 straggling ~9us.
"""

import numpy as np
from contextlib import ExitStack

CHUNK = 16
GROUPS = 8
EPS = 1e-5
B, C, H, W = 8, 1, 2048, 2048
D = 16
D_IN = CHUNK * CHUNK * C  # 256
HP = H // CHUNK  # 128 patch rows
WP = W // CHUNK  # 128 patch cols
P = 128
RB = 16  # 128-row blocks per image
N_CORES = 8

_CACHED_NC = {}


def _build_nc(mm_dtype="float32r"):
    import concourse.bass as bass
    import concourse.tile as tile
    from concourse import bacc, mybir

    f32 = mybir.dt.float32
    mmdt = getattr(mybir.dt, mm_dtype)
    nc = bacc.Bacc("TRN2", target_bir_lowering=False, debug=False,
                   num_devices=N_CORES)

    x = nc.dram_tensor("x", [H, W], mmdt, kind="ExternalInput").ap()
    # host-packed [p, s2, m] so the SBUF load is contiguous per partition
    wl = nc.dram_tensor("wl", [P, CHUNK * P], mmdt, kind="ExternalInput").ap()
    gmask = nc.dram_tensor("gmask", [P, P], f32, kind="ExternalInput").ap()
    consts = nc.dram_tensor("consts", [P, 3], f32, kind="ExternalInput").ap()
    # [p=(hblk,e), rg, w] flattened (rg = hi//8); host reorders to [D, HP, WP]
    out = nc.dram_tensor("out", [P, RB * WP], f32, kind="ExternalOutput").ap()

    Ident = mybir.ActivationFunctionType.Identity
    Sqrt = mybir.ActivationFunctionType.Sqrt
    Mult = mybir.AluOpType.mult
    Add = mybir.AluOpType.add
    Sub = mybir.AluOpType.subtract

    with tile.TileContext(nc) as tc, ExitStack() as ctx:
        const_pool = ctx.enter_context(tc.tile_pool(name="const", bufs=1))
        xin = ctx.enter_context(tc.tile_pool(name="xin", bufs=4))
        zpool = ctx.enter_context(tc.tile_pool(name="z", bufs=1))
        # 4 persistent PSUM banks hold z for all 16 blocks; 1 more for gp
        zpsum = ctx.enter_context(tc.tile_pool(name="zp", bufs=4, space="PSUM"))
        psg = ctx.enter_context(tc.tile_pool(name="psg", bufs=1, space="PSUM"))

        # x row-block rb covers image rows [rb*128, (rb+1)*128):
        # row = rb*128 + p, p = (hblk, s1)
        xr = x.rearrange("(rb p) c -> rb p c", rb=RB, p=P)

        def dma_block(eng, xt, src):
            # engine-15 relief: [0:112] sprays over all 16 engines, the two
            # 8-desc tails land on engines 0-7 only
            eng.dma_start(out=xt[0:112], in_=src[0:112])
            eng.dma_start(out=xt[112:120], in_=src[112:120])
            eng.dma_start(out=xt[120:128], in_=src[120:128])

        # stream: even blocks on SP ring, odd blocks + weights on ACT ring;
        # blocks {0,5,10,15} stay uniform so engines 0-7 don't overfill
        xts = []
        for rb in range(RB):
            eng = nc.sync if rb % 2 == 0 else nc.scalar
            xt = xin.tile([P, W], mmdt, tag="xt", name=f"xt{rb}")
            if rb == RB - 1:
                # last block in two w-halves so its matmuls start earlier
                h = W // 2
                eng.dma_start(out=xt[:, :h], in_=xr[rb, :, :h])
                eng.dma_start(out=xt[:, h:], in_=xr[rb, :, h:])
            elif rb % 5 == 0:
                eng.dma_start(out=xt, in_=xr[rb])
            else:
                dma_block(eng, xt, xr[rb])
            xts.append(xt)
            if rb == 0:
                wtile = const_pool.tile([P, CHUNK, P], mmdt)
                wv = wtile.rearrange("p s m -> p (s m)")
                dma_block(nc.scalar, wv, wl)
                gmt = const_pool.tile([P, P], f32)
                nc.scalar.dma_start(out=gmt, in_=gmask)
                cvt = const_pool.tile([P, 3], f32)
                nc.scalar.dma_start(out=cvt, in_=consts)
                epst = const_pool.tile([P, 1], f32)
                nc.vector.memset(epst, EPS)
                # touch Sqrt (exact scale/bias shape used later) early so its
                # ACT table loads during the stream, not in the stats chain
                warm = const_pool.tile([P, 1], f32)
                nc.scalar.activation(out=warm, in_=epst, func=Sqrt,
                                     bias=epst, scale=-1.0)
                nc.scalar.activation(out=warm, in_=epst, func=Ident,
                                     bias=epst, scale=epst)

        nstats = RB + 1  # last block contributes two half-stats
        statsall = zpool.tile([P, nstats, nc.vector.BN_STATS_DIM], f32)
        zts = [zpsum.tile([P, 4, WP], f32, tag="zt", name=f"zt{g}")
               for g in range(4)]

        for rb in range(RB):
            xt = xts[rb]
            xs = xt.rearrange("p (w s) -> p w s", s=CHUNK)
            pt = zts[rb // 4][:, rb % 4]
            if rb == RB - 1:
                hw = WP // 2
                for half in range(2):
                    ws = slice(half * hw, (half + 1) * hw)
                    for s2 in range(CHUNK):
                        nc.tensor.matmul(pt[:, ws], lhsT=wtile[:, s2, :],
                                         rhs=xs[:, ws, s2],
                                         start=(s2 == 0),
                                         stop=(s2 == CHUNK - 1))
                    nc.vector.bn_stats(out=statsall[:, rb + half],
                                       in_=pt[:, ws])
            else:
                for s2 in range(CHUNK):
                    nc.tensor.matmul(pt, lhsT=wtile[:, s2, :],
                                     rhs=xs[:, :, s2],
                                     start=(s2 == 0), stop=(s2 == CHUNK - 1))
                nc.vector.bn_stats(out=statsall[:, rb], in_=pt)

        # Per-partition raw mean'/var over all 2048 elements; fc bias b2 is
        # folded in here (z_true = raw + b2) and into the normalize offset.
        mv = zpool.tile([P, 2], f32)
        nc.vector.bn_aggr(out=mv, in_=statsall)
        # me2 = (-mean, E2) of biased z per partition:
        #   negmean = -(mean' + b2);  E2 = negmean^2 + var
        me2 = zpool.tile([P, 2], f32)
        nc.vector.tensor_scalar(out=me2[:, 0:1], in0=mv[:, 0:1],
                                scalar1=cvt[:, 0:1], scalar2=-1.0,
                                op0=Add, op1=Mult)
        nc.vector.scalar_tensor_tensor(
            out=me2[:, 1:2], in0=me2[:, 0:1], scalar=me2[:, 0:1],
            in1=mv[:, 1:2], op0=Mult, op1=Add)
        # Grouped cross-partition average + broadcast in one matmul:
        # gp[p'] = (1/16) * sum_{p in group(p')} me2[p] = (-mean_g, E2_g)
        gp = psg.tile([P, 2], f32)
        nc.tensor.matmul(gp, lhsT=gmt, rhs=me2, start=True, stop=True)
        gsb = zpool.tile([P, 2], f32)
        nc.vector.tensor_copy(gsb, gp)
        # negvar = mean_g^2 - E2_g;  sd = sqrt(-negvar + eps)
        negvar = zpool.tile([P, 1], f32)
        nc.vector.scalar_tensor_tensor(
            out=negvar, in0=gsb[:, 0:1], scalar=gsb[:, 0:1], in1=gsb[:, 1:2],
            op0=Mult, op1=Sub)
        sd = zpool.tile([P, 1], f32)
        nc.scalar.activation(out=sd, in_=negvar, func=Sqrt, bias=epst,
                             scale=-1.0)
        # v = b2 - mean_g (overlaps with Sqrt on the ACT engine)
        v = zpool.tile([P, 1], f32)
        nc.vector.tensor_add(v, cvt[:, 0:1], gsb[:, 0:1])
        rs = zpool.tile([P, 1], f32)
        nc.vector.reciprocal(rs, sd)
        # out = raw*A + B2 with A = rsqrt*gamma, B2 = (b2 - mean_g)*A + beta
        A = zpool.tile([P, 1], f32)
        nc.vector.tensor_mul(A, rs, cvt[:, 1:2])
        B2 = zpool.tile([P, 1], f32)
        nc.vector.scalar_tensor_tensor(
            out=B2, in0=A, scalar=v, in1=cvt[:, 2:3], op0=Mult, op1=Add)

        # normalize PSUM->SBUF in 4 chunks (ACT and DVE split them), each
        # chunk's output DMA overlapping the next chunk's normalize
        onorm = zpool.tile([P, RB * WP], f32)
        for g in range(4):
            sl = slice(g * 4 * WP, (g + 1) * 4 * WP)
            zin = zts[g].rearrange("p a b -> p (a b)")
            if g % 2 == 0:
                nc.vector.tensor_scalar(out=onorm[:, sl], in0=zin,
                                        scalar1=A, scalar2=B2,
                                        op0=Mult, op1=Add)
            else:
                nc.scalar.activation(out=onorm[:, sl], in_=zin,
                                     func=Ident, scale=A, bias=B2)
            eng = nc.sync if g % 2 == 0 else nc.scalar
            eng.dma_start(out=out[:, sl], in_=onorm[:, sl])

    nc.compile()
    return nc


def _host_weights(fc_wr, fc_wi, fc_br, fc_bi, conv_w, conv_b, gamma, beta):
    fc_wr = np.asarray(fc_wr, np.float64)
    fc_wi = np.asarray(fc_wi, np.float64)
    fc_br = np.asarray(fc_br, np.float64)
    fc_bi = np.asarray(fc_bi, np.float64)
    conv_w = np.asarray(conv_w, np.float64)
    conv_b = np.asarray(conv_b, np.float64)
    gamma = np.asarray(gamma, np.float64)
    beta = np.asarray(beta, np.float64)

    j = np.arange(D_IN)
    F = np.exp(-2j * np.pi * np.outer(j, j) / D_IN) / np.sqrt(D_IN)
    d = np.arange(D)
    G = np.exp(2j * np.pi * np.outer(d, d) / D) / np.sqrt(D)
    Wc = fc_wr + 1j * fc_wi
    bc = fc_br + 1j * fc_bi
    M2 = (np.real(F @ Wc.T @ G) @ conv_w.T).astype(np.float32)  # [256, 16]
    b2 = (np.real(bc @ G) @ conv_w.T + conv_b).astype(np.float32)  # [16]

    # Block-diagonal lhsT: wl[hblk*16+s1, s2, hblk*16+e] = M2[s1*16+s2, e],
    # packed [p, s2*128+m] for a contiguous per-partition SBUF load.
    wl = np.zeros((CHUNK, P, P), np.float32)  # [s2, p, m]
    blk = M2.reshape(CHUNK, CHUNK, D).transpose(1, 0, 2)  # [s2, s1, e]
    for hb in range(8):
        wl[:, hb * 16:hb * 16 + 16, hb * 16:hb * 16 + 16] = blk
    wl = np.ascontiguousarray(wl.transpose(1, 0, 2).reshape(P, CHUNK * P))

    # Group-average + broadcast mask; each partition holds 2048 elements,
    # each group spans 16 partitions -> scale 1/16 on the per-partition means
    pidx = np.arange(P)
    grp = (pidx % D) // (D // GROUPS)
    gmask = (grp[:, None] == grp[None, :]).astype(np.float32) / 16.0

    e = pidx % D
    consts = np.stack([b2[e], gamma.astype(np.float32)[e],
                       beta.astype(np.float32)[e]], axis=1)  # [128, 3]
    return wl, gmask, consts


def kernel(x, fc_wr, fc_wi, fc_br, fc_bi, conv_w, conv_b, gamma, beta,
           _return_results=False, _trace=False, _mm_dtype="float32r"):
    from concourse.bass_utils import run_bass_kernel_spmd

    if _mm_dtype not in _CACHED_NC:
        _CACHED_NC[_mm_dtype] = _build_nc(_mm_dtype)
    nc = _CACHED_NC[_mm_dtype]

    wl, gmask, consts = _host_weights(fc_wr, fc_wi, fc_br, fc_bi,
                                      conv_w, conv_b, gamma, beta)
    x = np.ascontiguousarray(np.asarray(x, np.float32).reshape(B, H, W))
    in_maps = [{"x": x[b], "wl": wl, "gmask": gmask, "consts": consts}
               for b in range(N_CORES)]
    res = run_bass_kernel_spmd(nc, in_maps, list(range(N_CORES)),
                               trace=_trace)
    # device layout [p=(hblk,e), rg, w] -> [D, HP, WP], hi = rg*8 + hblk
    out = np.stack(
        [res.results[b]["out"].reshape(8, D, RB, WP)
         .transpose(1, 2, 0, 3).reshape(D, HP, WP)
         for b in range(N_CORES)], axis=0)
    if _return_results:
        return out, res
    return out


# revision 7
# speedup vs baseline: 7.1932x; 1.1940x over previous
"""CFNO forward kernel for Trainium2 (8 NeuronCores, data-parallel over batch).

The reference computes, per 16x16 patch p (flattened to 256):
    fft = FFT_256(p) (ortho); fc = fft @ Wc^T + bc; y = Re(IFFT_16(fc)) (ortho)
    z = y @ conv_w^T + conv_b;  out = GroupNorm_8(z) * gamma + beta

Because p is real and every step before GroupNorm is linear, the whole chain
folds into one real matrix on the host:
    M2 = Re(F @ Wc^T @ G) @ conv_w^T   [256, 16]
    b2 = Re(bc @ G) @ conv_w^T + conv_b [16]
    z  = p @ M2 + b2
(F = symmetric 256-pt DFT matrix / sqrt(256); G = inverse 16-pt DFT / sqrt(16))

On-device per core (one batch image, x [2048, 2048]):
  - 16 row-blocks of 128 image rows; SBUF layout [128 part=(hblk, s1), 2048]
  - per block, 16 PSUM-accumulating matmuls (one per patch-column offset s2,
    free dim 128) with a block-diagonal lhsT so all 8 h-blocks share a matmul;
    float32r keeps full fp32 storage with a fast (TF32-like) PE mode
  - z stays in PSUM (4 banks hold all 16 blocks); bn_stats reads PSUM; the
    fc bias b2 is folded into the final normalize coefficients
  - one mask-matmul does the grouped cross-partition reduce AND broadcast
  - normalize (z*A + B) fused with the PSUM->SBUF move, chunked, with the
    output DMA of each chunk overlapping the next chunk's normalize

DMA scheduling (probed on HW): only full 128-partition transfers run at
line rate — partition-subset dma_starts (e.g. [0:112] + tails) drop to
~half the per-engine rate (port/engine misalignment), and non-divisible
partition counts like [0:127] degenerate onto a single engine.  So every
block is one uniform [128, 2048] dma_start (~417 GB/s marginal when
pipelined); the ~17%-slow engine 15 (known HW quirk) is left as-is.
"""

import numpy as np
from contextlib import ExitStack

CHUNK = 16
GROUPS = 8
EPS = 1e-5
B, C, H, W = 8, 1, 2048, 2048
D = 16
D_IN = CHUNK * CHUNK * C  # 256
HP = H // CHUNK  # 128 patch rows
WP = W // CHUNK  # 128 patch cols
P = 128
RB = 16  # 128-row blocks per image
N_CORES = 8

_CACHED_NC = {}


def _build_nc(mm_dtype="float32r"):
    import concourse.bass as bass
    import concourse.tile as tile
    from concourse import bacc, mybir

    f32 = mybir.dt.float32
    mmdt = getattr(mybir.dt, mm_dtype)
    nc = bacc.Bacc("TRN2", target_bir_lowering=False, debug=False,
                   num_devices=N_CORES)

    x = nc.dram_tensor("x", [H, W], mmdt, kind="ExternalInput").ap()
    # host-packed [p, s2, m] so the SBUF load is contiguous per partition
    wl = nc.dram_tensor("wl", [P, CHUNK * P], mmdt, kind="ExternalInput").ap()
    gmask = nc.dram_tensor("gmask", [P, P], f32, kind="ExternalInput").ap()
    consts = nc.dram_tensor("consts", [P, 3], f32, kind="ExternalInput").ap()
    # [p=(hblk,e), rg, w] flattened (rg = hi//8); host reorders to [D, HP, WP]
    out = nc.dram_tensor("out", [P, RB * WP], f32, kind="ExternalOutput").ap()

    Ident = mybir.ActivationFunctionType.Identity
    Sqrt = mybir.ActivationFunctionType.Sqrt
    Mult = mybir.AluOpType.mult
    Add = mybir.AluOpType.add
    Sub = mybir.AluOpType.subtract

    with tile.TileContext(nc) as tc, ExitStack() as ctx:
        const_pool = ctx.enter_context(tc.tile_pool(name="const", bufs=1))
        xin = ctx.enter_context(tc.tile_pool(name="xin", bufs=4))
        zpool = ctx.enter_context(tc.tile_pool(name="z", bufs=1))
        # 4 persistent PSUM banks hold z for all 16 blocks; 1 more for gp
        zpsum = ctx.enter_context(tc.tile_pool(name="zp", bufs=4, space="PSUM"))
        psg = ctx.enter_context(tc.tile_pool(name="psg", bufs=1, space="PSUM"))

        # x row-block rb covers image rows [rb*128, (rb+1)*128):
        # row = rb*128 + p, p = (hblk, s1)
        xr = x.rearrange("(rb p) c -> rb p c", rb=RB, p=P)

        # stream: x blocks on the SP ring, weights/consts on the ACT ring
        xts = []
        for rb in range(RB):
            xt = xin.tile([P, W], mmdt, tag="xt", name=f"xt{rb}")
            if rb == RB - 1:
                # last block in two w-halves so its matmuls start earlier
                h = W // 2
                nc.sync.dma_start(out=xt[:, :h], in_=xr[rb, :, :h])
                nc.sync.dma_start(out=xt[:, h:], in_=xr[rb, :, h:])
            else:
                nc.sync.dma_start(out=xt, in_=xr[rb])
            xts.append(xt)
            if rb == 0:
                wtile = const_pool.tile([P, CHUNK, P], mmdt)
                nc.scalar.dma_start(
                    out=wtile.rearrange("p s m -> p (s m)"), in_=wl)
                gmt = const_pool.tile([P, P], f32)
                nc.scalar.dma_start(out=gmt, in_=gmask)
                cvt = const_pool.tile([P, 3], f32)
                nc.scalar.dma_start(out=cvt, in_=consts)
                epst = const_pool.tile([P, 1], f32)
                nc.vector.memset(epst, EPS)
                # touch Sqrt (exact scale/bias shape used later) early so its
                # ACT table loads during the stream, not in the stats chain
                warm = const_pool.tile([P, 1], f32)
                nc.scalar.activation(out=warm, in_=epst, func=Sqrt,
                                     bias=epst, scale=-1.0)
                nc.scalar.activation(out=warm, in_=epst, func=Ident,
                                     bias=epst, scale=epst)

        nstats = RB + 1  # last block contributes two half-stats
        statsall = zpool.tile([P, nstats, nc.vector.BN_STATS_DIM], f32)
        zts = [zpsum.tile([P, 4, WP], f32, tag="zt", name=f"zt{g}")
               for g in range(4)]

        for rb in range(RB):
            xt = xts[rb]
            xs = xt.rearrange("p (w s) -> p w s", s=CHUNK)
            pt = zts[rb // 4][:, rb % 4]
            if rb == RB - 1:
                hw = WP // 2
                for half in range(2):
                    ws = slice(half * hw, (half + 1) * hw)
                    for s2 in range(CHUNK):
                        nc.tensor.matmul(pt[:, ws], lhsT=wtile[:, s2, :],
                                         rhs=xs[:, ws, s2],
                                         start=(s2 == 0),
                                         stop=(s2 == CHUNK - 1))
                    nc.vector.bn_stats(out=statsall[:, rb + half],
                                       in_=pt[:, ws])
            else:
                for s2 in range(CHUNK):
                    nc.tensor.matmul(pt, lhsT=wtile[:, s2, :],
                                     rhs=xs[:, :, s2],
                                     start=(s2 == 0), stop=(s2 == CHUNK - 1))
                nc.vector.bn_stats(out=statsall[:, rb], in_=pt)

        # Per-partition raw mean'/var over all 2048 elements; fc bias b2 is
        # folded in here (z_true = raw + b2) and into the normalize offset.
        mv = zpool.tile([P, 2], f32)
        nc.vector.bn_aggr(out=mv, in_=statsall)
        # me2 = (-mean, E2) of biased z per partition:
        #   negmean = -(mean' + b2);  E2 = negmean^2 + var
        me2 = zpool.tile([P, 2], f32)
        nc.vector.tensor_scalar(out=me2[:, 0:1], in0=mv[:, 0:1],
                                scalar1=cvt[:, 0:1], scalar2=-1.0,
                                op0=Add, op1=Mult)
        nc.vector.scalar_tensor_tensor(
            out=me2[:, 1:2], in0=me2[:, 0:1], scalar=me2[:, 0:1],
            in1=mv[:, 1:2], op0=Mult, op1=Add)
        # Grouped cross-partition average + broadcast in one matmul:
        # gp[p'] = (1/16) * sum_{p in group(p')} me2[p] = (-mean_g, E2_g)
        gp = psg.tile([P, 2], f32)
        nc.tensor.matmul(gp, lhsT=gmt, rhs=me2, start=True, stop=True)
        gsb = zpool.tile([P, 2], f32)
        nc.vector.tensor_copy(gsb, gp)
        # negvar = mean_g^2 - E2_g;  sd = sqrt(-negvar + eps)
        negvar = zpool.tile([P, 1], f32)
        nc.vector.scalar_tensor_tensor(
            out=negvar, in0=gsb[:, 0:1], scalar=gsb[:, 0:1], in1=gsb[:, 1:2],
            op0=Mult, op1=Sub)
        sd = zpool.tile([P, 1], f32)
        nc.scalar.activation(out=sd, in_=negvar, func=Sqrt, bias=epst,
                             scale=-1.0)
        # v = b2 - mean_g (overlaps with Sqrt on the ACT engine)
        v = zpool.tile([P, 1], f32)
        nc.vector.tensor_add(v, cvt[:, 0:1], gsb[:, 0:1])
        rs = zpool.tile([P, 1], f32)
        nc.vector.reciprocal(rs, sd)
        # out = raw*A + B2 with A = rsqrt*gamma, B2 = (b2 - mean_g)*A + beta
        A = zpool.tile([P, 1], f32)
        nc.vector.tensor_mul(A, rs, cvt[:, 1:2])
        B2 = zpool.tile([P, 1], f32)
        nc.vector.scalar_tensor_tensor(
            out=B2, in0=A, scalar=v, in1=cvt[:, 2:3], op0=Mult, op1=Add)

        # normalize PSUM->SBUF in 4 chunks (ACT and DVE split them), each
        # chunk's output DMA overlapping the next chunk's normalize
        onorm = zpool.tile([P, RB * WP], f32)
        for g in range(4):
            sl = slice(g * 4 * WP, (g + 1) * 4 * WP)
            zin = zts[g].rearrange("p a b -> p (a b)")
            if g % 2 == 0:
                nc.vector.tensor_scalar(out=onorm[:, sl], in0=zin,
                                        scalar1=A, scalar2=B2,
                                        op0=Mult, op1=Add)
            else:
                nc.scalar.activation(out=onorm[:, sl], in_=zin,
                                     func=Ident, scale=A, bias=B2)
            eng = nc.sync if g % 2 == 0 else nc.scalar
            eng.dma_start(out=out[:, sl], in_=onorm[:, sl])

    nc.compile()
    return nc


def _host_weights(fc_wr, fc_wi, fc_br, fc_bi, conv_w, conv_b, gamma, beta):
    fc_wr = np.asarray(fc_wr, np.float64)
    fc_wi = np.asarray(fc_wi, np.float64)
    fc_br = np.asarray(fc_br, np.float64)
    fc_bi = np.asarray(fc_bi, np.float64)
    conv_w = np.asarray(conv_w, np.float64)
    conv_b = np.asarray(conv_b, np.float64)
    gamma = np.asarray(gamma, np.float64)
    beta = np.asarray(beta, np.float64)

    j = np.arange(D_IN)
    F = np.exp(-2j * np.pi * np.outer(j, j) / D_IN) / np.sqrt(D_IN)
    d = np.arange(D)
    G = np.exp(2j * np.pi * np.outer(d, d) / D) / np.sqrt(D)
    Wc = fc_wr + 1j * fc_wi
    bc = fc_br + 1j * fc_bi
    M2 = (np.real(F @ Wc.T @ G) @ conv_w.T).astype(np.float32)  # [256, 16]
    b2 = (np.real(bc @ G) @ conv_w.T + conv_b).astype(np.float32)  # [16]

    # Block-diagonal lhsT: wl[hblk*16+s1, s2, hblk*16+e] = M2[s1*16+s2, e],
    # packed [p, s2*128+m] for a contiguous per-partition SBUF load.
    wl = np.zeros((CHUNK, P, P), np.float32)  # [s2, p, m]
    blk = M2.reshape(CHUNK, CHUNK, D).transpose(1, 0, 2)  # [s2, s1, e]
    for hb in range(8):
        wl[:, hb * 16:hb * 16 + 16, hb * 16:hb * 16 + 16] = blk
    wl = np.ascontiguousarray(wl.transpose(1, 0, 2).reshape(P, CHUNK * P))

    # Group-average + broadcast mask; each partition holds 2048 elements,
    # each group spans 16 partitions -> scale 1/16 on the per-partition means
    pidx = np.arange(P)
    grp = (pidx % D) // (D // GROUPS)
    gmask = (grp[:, None] == grp[None, :]).astype(np.float32) / 16.0

    e = pidx % D
    consts = np.stack([b2[e], gamma.astype(np.float32)[e],
                       beta.astype(np.float32)[e]], axis=1)  # [128, 3]
    return wl, gmask, consts


def kernel(x, fc_wr, fc_wi, fc_br, fc_bi, conv_w, conv_b, gamma, beta,
           _return_results=False, _trace=False, _mm_dtype="float32r"):
    from concourse.bass_utils import run_bass_kernel_spmd

    if _mm_dtype not in _CACHED_NC:
        _CACHED_NC[_mm_dtype] = _build_nc(_mm_dtype)
    nc = _CACHED_NC[_mm_dtype]

    wl, gmask, consts = _host_weights(fc_wr, fc_wi, fc_br, fc_bi,
                                      conv_w, conv_b, gamma, beta)
    x = np.ascontiguousarray(np.asarray(x, np.float32).reshape(B, H, W))
    in_maps = [{"x": x[b], "wl": wl, "gmask": gmask, "consts": consts}
               for b in range(N_CORES)]
    res = run_bass_kernel_spmd(nc, in_maps, list(range(N_CORES)),
                               trace=_trace)
    # device layout [p=(hblk,e), rg, w] -> [D, HP, WP], hi = rg*8 + hblk
    out = np.stack(
        [res.results[b]["out"].reshape(8, D, RB, WP)
         .transpose(1, 2, 0, 3).reshape(D, HP, WP)
         for b in range(N_CORES)], axis=0)
    if _return_results:
        return out, res
    return out
